# revision 13
# baseline (speedup 1.0000x reference)
"""Sparse-weight matmul (BiologicalModule) on 8 Trainium2 NeuronCores.

Computes: out = tanh(x @ scatter_coo(kernel_vector, nonzero_ind) + bias)
  x [32, 30000] f32, 500K COO nonzeros into a [30000, 2048] weight matrix.

Strategy (units-sharded, 256 output columns per core):
  - Never materialize the dense [30000, 2048] weight matrix. In CSC view,
    out_T[c, :] = sum_k v[c,k] * x[:, r[c,k]].
  - kernel() packs, per core, a padded-CSC payload: column c's k-th entry
    value (vals[c, k]) and the x column-vector it touches (g[c, b, k]),
    columns on SBUF partitions, k innermost/contiguous. Columns here have
    244-245 entries, so kp=248 wastes only 1.4%. This is pure data layout /
    sharding prep - no arithmetic happens on host.
  - The ~4 MB/core fp16 payload streams in batch-row chunks whose
    partition-lines stay >= 512 B contiguous (full DMA model rate), every
    chunk with its own SBUF buffer so the DMA stream never stalls on
    compute. The kernel is then DMA-bound; per chunk the batch rows are
    split across three engines, each kept under the ~11.6 us stream time:
      * Pool (GPSIMD) rows: one fused scalar_tensor_tensor (multiply +
        accum_out row-sum) straight from the g tile,
      * DVE rows: one 2x-mode fp16 tensor_tensor multiply per chunk, then
        a 4x-mode tensor_scalar accum_out reduce per row,
      * ACT rows: activation(Copy) with accum_out on DVE's product,
    and ACT applies fused bias+tanh per output group, shipped in 3 DMAs so
    only the last 4 rows sit in the pipeline tail.
"""

import sys

import numpy as np

_TRN_REPO = "/opt/trn_rl_repo"
if _TRN_REPO not in sys.path:
    sys.path.insert(0, _TRN_REPO)

INPUT_DIM = 30000
UNITS = 2048
BATCH = 32
N_CORES = 8
UNITS_PER_CORE = UNITS // N_CORES  # 256
BLOCKS_PER_CORE = UNITS_PER_CORE // 128  # 2

# Stream chunks: (block, b0, b1, nP, nPA, nDA). Per chunk, rows are routed:
#   nP   Pool multiply (batched tensor_tensor) -> DVE 4x tensor_scalar accum
#   nPA  Pool multiply -> ACT activation accum
#   nDA  DVE feeder multiply -> ACT activation accum
#   rest DVE multiply + 4x tensor_scalar accum
# (assignment found by a pipeline-model search minimizing end-to-end time;
# the small tail chunks are DVE-solo to keep the pipeline tail short)
CHUNKS = [
    (0, 0, 8, 1, 1, 0),
    (1, 0, 8, 2, 0, 4),
    (0, 8, 16, 0, 2, 0),
    (1, 8, 16, 0, 2, 0),
    (0, 16, 24, 2, 0, 3),
    (1, 16, 24, 1, 2, 0),
    (0, 24, 28, 0, 0, 2),
    (0, 28, 32, 2, 0, 0),
    (1, 24, 28, 0, 0, 0),
    (1, 28, 30, 2, 0, 0),
    (1, 30, 32, 2, 0, 0),
]
# Output groups (block, b0, b1) in readiness order; each is tanh'ed and
# DMA'd as soon as its accumulator slice completes.
OUT_GROUPS = [(1, 0, 24), (0, 0, 32), (1, 24, 32)]

_PROGRAM_CACHE = {}


def _build_program(kp):
    """Build + compile the SPMD bass program for padded column length kp."""
    from concourse import bacc, tile
    from concourse.bass import AP
    import concourse.mybir as mybir

    assert kp % 8 == 0
    f32 = mybir.dt.float32
    f16 = mybir.dt.float16
    pw = BLOCKS_PER_CORE * kp + 4  # vals for both blocks + bias (2xf32)

    nc = bacc.Bacc("TRN2", target_bir_lowering=False, debug=False,
                   num_devices=N_CORES)
    g_d = nc.dram_tensor("gv", [BLOCKS_PER_CORE, 128, BATCH, kp], f16,
                         kind="ExternalInput")
    vb_d = nc.dram_tensor("vb", [128, pw], f16, kind="ExternalInput")
    out_d = nc.dram_tensor("out", [128, BLOCKS_PER_CORE, BATCH], f32,
                           kind="ExternalOutput")

    with tile.TileContext(nc) as tc:
        with (
            tc.tile_pool(name="persist", bufs=1) as persist,
            tc.tile_pool(name="gwork", bufs=1) as gwork,
            tc.tile_pool(name="pwork", bufs=1) as pwork,
            tc.tile_pool(name="dscr", bufs=3) as dscr,
            tc.tile_pool(name="ascr", bufs=3) as ascr,
        ):
            vb_t = persist.tile([128, pw], f16, tag="vb")
            vals = [vb_t[:, blk * kp:(blk + 1) * kp]
                    for blk in range(BLOCKS_PER_CORE)]
            bias = vb_t[:, BLOCKS_PER_CORE * kp:].bitcast(f32)  # [128, 2]
            pt = [persist.tile([128, BATCH], f32, tag=f"pt{blk}",
                               name=f"pt{blk}")
                  for blk in range(BLOCKS_PER_CORE)]
            outp = persist.tile([128, BLOCKS_PER_CORE, BATCH], f32,
                                tag="outp")
            warm = persist.tile([128, 1], f32, tag="warm")

            # All payload DMAs up front, in stream order (HWDGE serializes;
            # unique buffers so no issue ever waits on compute). vb rides
            # second so the g stream starts immediately.
            g_tiles = []
            for i, (blk, b0, b1, nP, nPA, nDA) in enumerate(CHUNKS):
                g_t = gwork.tile([128, b1 - b0, kp], f16, tag=f"g{i}",
                                 name=f"g{i}")
                nc.sync.dma_start(g_t[:], g_d[blk, :, b0:b1])
                g_tiles.append(g_t)
                if i == 0:
                    nc.sync.dma_start(vb_t[:], vb_d[:])

            # Preload the ACT function table far off the critical path.
            nc.scalar.activation(warm[:], vb_t[:, 0:1],
                                 mybir.ActivationFunctionType.Tanh)

            def dve_acc(src, acc, name):
                scr = dscr.tile([128, kp], f16, tag="dscr", name=name)
                nc.vector.tensor_scalar(
                    scr[:], src, 1.0, 0.0,
                    mybir.AluOpType.mult, mybir.AluOpType.add,
                    accum_out=acc)

            def act_acc(src, acc):
                scr = ascr.tile([128, kp], f16, tag="ascr")
                nc.scalar.activation(
                    scr[:], src, mybir.ActivationFunctionType.Copy,
                    accum_out=acc)

            for i, (blk, b0, b1, nP, nPA, nDA) in enumerate(CHUNKS):
                g_t = g_tiles[i]
                nb = b1 - b0
                base = vals[blk]
                npool = nP + nPA
                prodp = None
                if npool > 0:
                    # batched Pool multiply covering its nP + nPA rows
                    prodp = pwork.tile([128, npool, kp], f16,
                                       tag=f"prodp{i}", name=f"prodp{i}")
                    v_bk = AP(base.tensor, base.offset,
                              [base.ap[0], [0, npool], base.ap[1]])
                    nc.gpsimd.tensor_tensor(prodp[:], g_t[:, :npool], v_bk,
                                            mybir.AluOpType.mult)
                    for j in range(nPA):
                        act_acc(prodp[:, nP + j],
                                pt[blk][:, b0 + nP + j:b0 + nP + j + 1])
                nd = nb - npool
                if nd > 0:
                    prod = pwork.tile([128, nd, kp], f16, tag=f"prod{i}",
                                      name=f"prod{i}")
                    if nDA > 0:
                        # feeder multiply for ACT rows, its own op so ACT
                        # starts without waiting on DVE-solo work
                        v_bk = AP(base.tensor, base.offset,
                                  [base.ap[0], [0, nDA], base.ap[1]])
                        nc.vector.tensor_tensor(
                            prod[:, :nDA], g_t[:, npool:npool + nDA], v_bk,
                            mybir.AluOpType.mult)
                        for j in range(nDA):
                            b = b0 + npool + j
                            act_acc(prod[:, j], pt[blk][:, b:b + 1])
                    nD = nd - nDA
                    if nD > 0:
                        v_bk = AP(base.tensor, base.offset,
                                  [base.ap[0], [0, nD], base.ap[1]])
                        nc.vector.tensor_tensor(
                            prod[:, nDA:], g_t[:, npool + nDA:], v_bk,
                            mybir.AluOpType.mult)
                        for j in range(nDA, nd):
                            b = b0 + npool + j
                            dve_acc(prod[:, j], pt[blk][:, b:b + 1],
                                    f"ds{i}_{j}")
                # DVE accums for the Pool-multiplied rows, queued last so
                # they don't head-of-line block DVE's own chunk work
                for j in range(nP):
                    dve_acc(prodp[:, j], pt[blk][:, b0 + j:b0 + j + 1],
                            f"dp{i}_{j}")

            for blk, b0, b1 in OUT_GROUPS:
                nc.scalar.activation(
                    outp[:, blk, b0:b1], pt[blk][:, b0:b1],
                    mybir.ActivationFunctionType.Tanh,
                    bias=bias[:, blk:blk + 1],
                )
                nc.sync.dma_start(out_d[:, blk, b0:b1], outp[:, blk, b0:b1])
    nc.compile()
    return nc


def _prepare(x, kernel_vector, bias, nonzero_ind):
    """Host-side shard prep (pure layout). Returns (kp, per-core inputs)."""
    x = np.asarray(x, dtype=np.float32)
    v = np.asarray(kernel_vector, dtype=np.float32).ravel()
    bias = np.asarray(bias, dtype=np.float32).ravel()
    ind = np.asarray(nonzero_ind)
    r = ind[:, 0].astype(np.int64)
    c = ind[:, 1].astype(np.int64)

    # COO .set semantics: de-duplicate (row, col), keeping the last occurrence.
    flat = r * UNITS + c
    if len(np.unique(flat)) != len(flat):
        _, last_rev = np.unique(flat[::-1], return_index=True)
        keep = np.sort(len(flat) - 1 - last_rev)
        r, c, v = r[keep], c[keep], v[keep]

    xt16 = np.ascontiguousarray(x.T).astype(np.float16)  # [INPUT_DIM, BATCH]

    # Sort by column, assign each entry its slot k within its column.
    order = np.argsort(c, kind="stable")
    r_s, c_s, v_s = r[order], c[order], v[order]
    counts = np.bincount(c_s, minlength=UNITS)
    kp = max(8, int(-(-counts.max() // 8)) * 8)
    starts = np.zeros(UNITS + 1, dtype=np.int64)
    np.cumsum(counts, out=starts[1:])
    k_s = np.arange(len(c_s), dtype=np.int64) - starts[c_s]

    # Padded-CSC payload, k innermost: g_all[c, b, k] = x[b, r[c,k]] fp16,
    # vals[c, k] fp16; padding slots stay 0.
    val_all = np.zeros((UNITS, kp), dtype=np.float16)
    val_all[c_s, k_s] = v_s.astype(np.float16)
    g_all = np.zeros((UNITS, BATCH, kp), dtype=np.float16)
    g_all[c_s, :, k_s] = xt16[r_s]

    g_all = g_all.reshape(N_CORES, BLOCKS_PER_CORE, 128, BATCH, kp)
    val_all = val_all.reshape(N_CORES, BLOCKS_PER_CORE, 128, kp)
    bias2 = bias.reshape(N_CORES, BLOCKS_PER_CORE, 128)

    pw = BLOCKS_PER_CORE * kp + 4
    in_maps = []
    for d in range(N_CORES):
        vb = np.zeros((128, pw), dtype=np.float16)
        for blk in range(BLOCKS_PER_CORE):
            vb[:, blk * kp:(blk + 1) * kp] = val_all[d, blk]
        vb[:, BLOCKS_PER_CORE * kp:] = np.ascontiguousarray(
            bias2[d].T.astype(np.float32)).view(np.float16)
        in_maps.append({
            "gv": g_all[d],
            "vb": vb,
        })
    return kp, in_maps


def _run(inputs, trace=False):
    from concourse.bass_utils import run_bass_kernel_spmd

    kp, in_maps = _prepare(**inputs)
    if kp not in _PROGRAM_CACHE:
        _PROGRAM_CACHE[kp] = _build_program(kp)
    nc = _PROGRAM_CACHE[kp]
    res = None
    for attempt in range(3):
        try:
            res = run_bass_kernel_spmd(
                nc, in_maps, list(range(N_CORES)), trace=trace,
            )
            break
        except Exception:
            # Transient device faults (e.g. NRT_EXEC_UNIT_UNRECOVERABLE)
            # clear on re-execution; re-raise only if persistent.
            if attempt == 2:
                raise
    assert res is not None
    # out per core: [128, BLOCKS_PER_CORE, BATCH] -> [256, BATCH]
    out_t = np.concatenate(
        [res.results[d]["out"].reshape(128, BLOCKS_PER_CORE, BATCH)
         .transpose(1, 0, 2).reshape(UNITS_PER_CORE, BATCH)
         for d in range(N_CORES)], axis=0)  # [2048, 32]
    out = np.ascontiguousarray(out_t.T).astype(np.float32)  # [32, 2048]
    return out, res


def kernel(**inputs):
    out, _ = _run(inputs, trace=False)
    return out


# revision 17
# speedup vs baseline: 1.0054x; 1.0054x over previous
"""Sparse-weight matmul (BiologicalModule) on 8 Trainium2 NeuronCores.

Computes: out = tanh(x @ scatter_coo(kernel_vector, nonzero_ind) + bias)
  x [32, 30000] f32, 500K COO nonzeros into a [30000, 2048] weight matrix.

Strategy (units-sharded, 256 output columns per core):
  - Never materialize the dense [30000, 2048] weight matrix. In CSC view,
    out_T[c, :] = sum_k v[c,k] * x[:, r[c,k]].
  - kernel() packs, per core, a padded-CSC payload: column c's k-th entry
    value (vals[c, k]) and the x column-vector it touches (g[c, b, k]),
    columns on SBUF partitions, k innermost/contiguous. Columns here have
    244-245 entries, so kp=248 wastes only 1.4%. This is pure data layout /
    sharding prep - no arithmetic happens on host.
  - The ~4 MB/core fp16 payload streams in batch-row chunks whose
    partition-lines stay >= 512 B contiguous (full DMA model rate), every
    chunk with its own SBUF buffer so the DMA stream never stalls on
    compute. The kernel is then DMA-bound; per chunk the batch rows are
    split across three engines, each kept under the ~11.6 us stream time:
      * Pool (GPSIMD) rows: one fused scalar_tensor_tensor (multiply +
        accum_out row-sum) straight from the g tile,
      * DVE rows: one 2x-mode fp16 tensor_tensor multiply per chunk, then
        a 4x-mode tensor_scalar accum_out reduce per row,
      * ACT rows: activation(Copy) with accum_out on DVE's product,
    and ACT applies fused bias+tanh per output group, shipped in 3 DMAs so
    only the last 4 rows sit in the pipeline tail.
"""

import sys

import numpy as np

_TRN_REPO = "/opt/trn_rl_repo"
if _TRN_REPO not in sys.path:
    sys.path.insert(0, _TRN_REPO)

INPUT_DIM = 30000
UNITS = 2048
BATCH = 32
N_CORES = 8
UNITS_PER_CORE = UNITS // N_CORES  # 256
BLOCKS_PER_CORE = UNITS_PER_CORE // 128  # 2

# Stream chunks: (block, b0, b1, nP, nPA, nDA, nPL). Per chunk, rows are
# routed, in this order:
#   nP   Pool multiply (batched tensor_tensor) -> DVE 4x tensor_scalar accum
#   nPA  Pool multiply -> ACT activation accum
#   nDA  DVE feeder multiply -> ACT activation accum
#   nPL  DVE feeder multiply -> Pool half-fold add -> ACT accum on 124
#   rest DVE multiply + 4x tensor_scalar accum
# (assignment found by a pipeline-model search minimizing end-to-end time;
# small first chunks start the pipeline earlier, small DVE-solo tail chunks
# keep the pipeline tail short)
CHUNKS = [
    (0, 0, 4, 0, 0, 0, 0),
    (1, 0, 4, 1, 0, 0, 1),
    (0, 4, 12, 2, 0, 3, 1),
    (1, 4, 12, 0, 2, 0, 2),
    (0, 12, 20, 0, 2, 0, 0),
    (1, 12, 20, 0, 3, 0, 0),
    (0, 20, 28, 0, 2, 0, 0),
    (1, 20, 28, 2, 1, 0, 0),
    (0, 28, 32, 1, 0, 0, 0),
    (1, 28, 30, 0, 0, 0, 0),
    (1, 30, 32, 2, 0, 0, 0),
]
# Output groups (block, b0, b1) in readiness order; each is tanh'ed and
# DMA'd as soon as its accumulator slice completes.
OUT_GROUPS = [(1, 0, 24), (0, 0, 32), (1, 24, 32)]

_PROGRAM_CACHE = {}


def _build_program(kp):
    """Build + compile the SPMD bass program for padded column length kp."""
    from concourse import bacc, tile
    from concourse.bass import AP
    import concourse.mybir as mybir

    assert kp % 8 == 0
    f32 = mybir.dt.float32
    f16 = mybir.dt.float16
    pw = BLOCKS_PER_CORE * kp + 4  # vals for both blocks + bias (2xf32)

    nc = bacc.Bacc("TRN2", target_bir_lowering=False, debug=False,
                   num_devices=N_CORES)
    g_d = nc.dram_tensor("gv", [BLOCKS_PER_CORE, 128, BATCH, kp], f16,
                         kind="ExternalInput")
    vb_d = nc.dram_tensor("vb", [128, pw], f16, kind="ExternalInput")
    out_d = nc.dram_tensor("out", [128, BLOCKS_PER_CORE, BATCH], f32,
                           kind="ExternalOutput")

    with tile.TileContext(nc) as tc:
        with (
            tc.tile_pool(name="persist", bufs=1) as persist,
            tc.tile_pool(name="gwork", bufs=1) as gwork,
            tc.tile_pool(name="pwork", bufs=1) as pwork,
            tc.tile_pool(name="dscr", bufs=3) as dscr,
            tc.tile_pool(name="ascr", bufs=3) as ascr,
        ):
            vb_t = persist.tile([128, pw], f16, tag="vb")
            vals = [vb_t[:, blk * kp:(blk + 1) * kp]
                    for blk in range(BLOCKS_PER_CORE)]
            bias = vb_t[:, BLOCKS_PER_CORE * kp:].bitcast(f32)  # [128, 2]
            pt = [persist.tile([128, BATCH], f32, tag=f"pt{blk}",
                               name=f"pt{blk}")
                  for blk in range(BLOCKS_PER_CORE)]
            outp = persist.tile([128, BLOCKS_PER_CORE, BATCH], f32,
                                tag="outp")
            warm = persist.tile([128, 1], f32, tag="warm")

            # All payload DMAs up front, in stream order (HWDGE serializes;
            # unique buffers so no issue ever waits on compute). vb rides
            # second so the g stream starts immediately.
            g_tiles = []
            for i, chunk in enumerate(CHUNKS):
                blk, b0, b1 = chunk[:3]
                g_t = gwork.tile([128, b1 - b0, kp], f16, tag=f"g{i}",
                                 name=f"g{i}")
                nc.sync.dma_start(g_t[:], g_d[blk, :, b0:b1])
                g_tiles.append(g_t)
                if i == 0:
                    nc.sync.dma_start(vb_t[:], vb_d[:])

            # Preload the ACT function table far off the critical path.
            nc.scalar.activation(warm[:], vb_t[:, 0:1],
                                 mybir.ActivationFunctionType.Tanh)

            def dve_acc(src, acc, name):
                scr = dscr.tile([128, kp], f16, tag="dscr", name=name)
                nc.vector.tensor_scalar(
                    scr[:], src, 1.0, 0.0,
                    mybir.AluOpType.mult, mybir.AluOpType.add,
                    accum_out=acc)

            def act_acc(src, acc):
                scr = ascr.tile([128, kp], f16, tag="ascr")
                nc.scalar.activation(
                    scr[:, :src.shape[-1]], src,
                    mybir.ActivationFunctionType.Copy,
                    accum_out=acc)

            half = kp // 2
            for i, (blk, b0, b1, nP, nPA, nDA, nPL) in enumerate(CHUNKS):
                g_t = g_tiles[i]
                nb = b1 - b0
                base = vals[blk]
                npool = nP + nPA
                prodp = None
                if npool > 0:
                    # batched Pool multiply covering its nP + nPA rows
                    prodp = pwork.tile([128, npool, kp], f16,
                                       tag=f"prodp{i}", name=f"prodp{i}")
                    v_bk = AP(base.tensor, base.offset,
                              [base.ap[0], [0, npool], base.ap[1]])
                    nc.gpsimd.tensor_tensor(prodp[:], g_t[:, :npool], v_bk,
                                            mybir.AluOpType.mult)
                    for j in range(nPA):
                        act_acc(prodp[:, nP + j],
                                pt[blk][:, b0 + nP + j:b0 + nP + j + 1])
                nd = nb - npool
                if nd > 0:
                    prod = pwork.tile([128, nd, kp], f16, tag=f"prod{i}",
                                      name=f"prod{i}")
                    if nDA > 0:
                        # feeder multiply for ACT rows, its own op so ACT
                        # starts without waiting on DVE-solo work
                        v_bk = AP(base.tensor, base.offset,
                                  [base.ap[0], [0, nDA], base.ap[1]])
                        nc.vector.tensor_tensor(
                            prod[:, :nDA], g_t[:, npool:npool + nDA], v_bk,
                            mybir.AluOpType.mult)
                        for j in range(nDA):
                            b = b0 + npool + j
                            act_acc(prod[:, j], pt[blk][:, b:b + 1])
                    if nPL > 0:
                        # feeder multiply, Pool folds the k-halves, ACT
                        # accumulates the half-length rows
                        v_bk = AP(base.tensor, base.offset,
                                  [base.ap[0], [0, nPL], base.ap[1]])
                        sl = prod[:, nDA:nDA + nPL]
                        nc.vector.tensor_tensor(
                            sl, g_t[:, npool + nDA:npool + nDA + nPL], v_bk,
                            mybir.AluOpType.mult)
                        with nc.allow_low_precision(
                                "fp16 half-fold; f32 accum"):
                            nc.gpsimd.tensor_tensor(
                                sl[:, :, :half], sl[:, :, :half],
                                sl[:, :, half:], mybir.AluOpType.add)
                        for j in range(nPL):
                            b = b0 + npool + nDA + j
                            act_acc(prod[:, nDA + j, :half],
                                    pt[blk][:, b:b + 1])
                    nD = nd - nDA - nPL
                    if nD > 0:
                        v_bk = AP(base.tensor, base.offset,
                                  [base.ap[0], [0, nD], base.ap[1]])
                        nc.vector.tensor_tensor(
                            prod[:, nDA + nPL:], g_t[:, npool + nDA + nPL:],
                            v_bk, mybir.AluOpType.mult)
                        for j in range(nDA + nPL, nd):
                            b = b0 + npool + j
                            dve_acc(prod[:, j], pt[blk][:, b:b + 1],
                                    f"ds{i}_{j}")
                # DVE accums for the Pool-multiplied rows, queued last so
                # they don't head-of-line block DVE's own chunk work
                for j in range(nP):
                    dve_acc(prodp[:, j], pt[blk][:, b0 + j:b0 + j + 1],
                            f"dp{i}_{j}")

            for blk, b0, b1 in OUT_GROUPS:
                nc.scalar.activation(
                    outp[:, blk, b0:b1], pt[blk][:, b0:b1],
                    mybir.ActivationFunctionType.Tanh,
                    bias=bias[:, blk:blk + 1],
                )
                nc.sync.dma_start(out_d[:, blk, b0:b1], outp[:, blk, b0:b1])
    nc.compile()
    return nc


def _prepare(x, kernel_vector, bias, nonzero_ind):
    """Host-side shard prep (pure layout). Returns (kp, per-core inputs)."""
    x = np.asarray(x, dtype=np.float32)
    v = np.asarray(kernel_vector, dtype=np.float32).ravel()
    bias = np.asarray(bias, dtype=np.float32).ravel()
    ind = np.asarray(nonzero_ind)
    r = ind[:, 0].astype(np.int64)
    c = ind[:, 1].astype(np.int64)

    # COO .set semantics: de-duplicate (row, col), keeping the last occurrence.
    flat = r * UNITS + c
    if len(np.unique(flat)) != len(flat):
        _, last_rev = np.unique(flat[::-1], return_index=True)
        keep = np.sort(len(flat) - 1 - last_rev)
        r, c, v = r[keep], c[keep], v[keep]

    xt16 = np.ascontiguousarray(x.T).astype(np.float16)  # [INPUT_DIM, BATCH]

    # Sort by column, assign each entry its slot k within its column.
    order = np.argsort(c, kind="stable")
    r_s, c_s, v_s = r[order], c[order], v[order]
    counts = np.bincount(c_s, minlength=UNITS)
    kp = max(8, int(-(-counts.max() // 8)) * 8)
    starts = np.zeros(UNITS + 1, dtype=np.int64)
    np.cumsum(counts, out=starts[1:])
    k_s = np.arange(len(c_s), dtype=np.int64) - starts[c_s]

    # Padded-CSC payload, k innermost: g_all[c, b, k] = x[b, r[c,k]] fp16,
    # vals[c, k] fp16; padding slots stay 0.
    val_all = np.zeros((UNITS, kp), dtype=np.float16)
    val_all[c_s, k_s] = v_s.astype(np.float16)
    g_all = np.zeros((UNITS, BATCH, kp), dtype=np.float16)
    g_all[c_s, :, k_s] = xt16[r_s]

    g_all = g_all.reshape(N_CORES, BLOCKS_PER_CORE, 128, BATCH, kp)
    val_all = val_all.reshape(N_CORES, BLOCKS_PER_CORE, 128, kp)
    bias2 = bias.reshape(N_CORES, BLOCKS_PER_CORE, 128)

    pw = BLOCKS_PER_CORE * kp + 4
    in_maps = []
    for d in range(N_CORES):
        vb = np.zeros((128, pw), dtype=np.float16)
        for blk in range(BLOCKS_PER_CORE):
            vb[:, blk * kp:(blk + 1) * kp] = val_all[d, blk]
        vb[:, BLOCKS_PER_CORE * kp:] = np.ascontiguousarray(
            bias2[d].T.astype(np.float32)).view(np.float16)
        in_maps.append({
            "gv": g_all[d],
            "vb": vb,
        })
    return kp, in_maps


def _run(inputs, trace=False):
    from concourse.bass_utils import run_bass_kernel_spmd

    kp, in_maps = _prepare(**inputs)
    if kp not in _PROGRAM_CACHE:
        _PROGRAM_CACHE[kp] = _build_program(kp)
    nc = _PROGRAM_CACHE[kp]
    res = None
    for attempt in range(3):
        try:
            res = run_bass_kernel_spmd(
                nc, in_maps, list(range(N_CORES)), trace=trace,
            )
            break
        except Exception:
            # Transient device faults (e.g. NRT_EXEC_UNIT_UNRECOVERABLE)
            # clear on re-execution; re-raise only if persistent.
            if attempt == 2:
                raise
    assert res is not None
    # out per core: [128, BLOCKS_PER_CORE, BATCH] -> [256, BATCH]
    out_t = np.concatenate(
        [res.results[d]["out"].reshape(128, BLOCKS_PER_CORE, BATCH)
         .transpose(1, 0, 2).reshape(UNITS_PER_CORE, BATCH)
         for d in range(N_CORES)], axis=0)  # [2048, 32]
    out = np.ascontiguousarray(out_t.T).astype(np.float32)  # [32, 2048]
    return out, res


def kernel(**inputs):
    out, _ = _run(inputs, trace=False)
    return out
